# revision 57
# baseline (speedup 1.0000x reference)
"""Multi-head causal attention (B=4, S=2048, D=1024, H=16) for 8 Trainium2 cores.

Sharding: core c = (batch b = c//2, head-group g = c%2). Each core computes,
for its batch and its 8 heads: QKV projections, causal softmax attention, and
a partial output projection (its heads' rows of Wo). Host sums the two
head-group partials per batch and adds the output bias.

v2 design notes (cost-model driven):
 - Attention matmuls (scores, diag masks, PV) run in bf16 so narrow
   (N<256) matmuls still run at 1 cycle/row; projections stay float32r.
 - Scores computed transposed, ST[k, q], with EXACT causal trims
   (512/384/256/128 widths); only the 128-wide diagonal block needs a
   mask, applied by one extra bf16 matmul per head (atri @ bb).
 - PV stationary per head is [ones(64) | V(64)], M=128: psum rows 0-63
   get the softmax denominator replicated 64x, rows 64-127 the context.
   Normalization is then just reciprocal_approx_fast on rows 0-63 plus a
   mixed-base-partition DVE multiply (verified exact on HW) - no gpsimd
   broadcast, no partition-shift DMAs.
 - Single software-pipelined phase: K/V/Q projections for q-tile qt+1 and
   the deferred output projection of qt-1 are emitted between attention
   c-slices of qt, so the PE has independent work whenever PV waits on
   the (Activation-engine-bound) exp stream.
"""

import sys

if "/opt/trn_rl_repo" not in sys.path:
    sys.path.insert(0, "/opt/trn_rl_repo")

import numpy as np
import ml_dtypes

B, S, D = 4, 2048, 1024
H, DH = 16, 64
NCORES = 8
GH = H // 2            # heads per core
GW = GH * DH           # head-group width (512)
NP = GW // 128         # head pairs per core (4)
SM_SCALE = float(1.0 / np.sqrt(np.float32(D)))
BIG = 1.0e30
BF = ml_dtypes.bfloat16


def build_mha_kernel(S_, D_, debug=False):
    import concourse.bass as bass  # noqa: F401
    import concourse.mybir as mybir
    import concourse.tile as tile
    from concourse import bacc

    f32 = mybir.dt.float32
    f32r = mybir.dt.float32r
    bf16 = mybir.dt.bfloat16

    KT = D_ // 128          # input-dim tiles
    NQT = S_ // 512         # q tiles
    NKC = S_ // 128         # key chunks

    nc = bacc.Bacc("TRN2", target_bir_lowering=False, debug=debug)

    XT_d = nc.dram_tensor("XT", [D_, S_], bf16, kind="ExternalInput")
    WQ_d = nc.dram_tensor("WQ", [D_, GW], bf16, kind="ExternalInput")
    WK_d = nc.dram_tensor("WK", [D_, GW], bf16, kind="ExternalInput")
    WV_d = nc.dram_tensor("WV", [D_, GW], bf16, kind="ExternalInput")
    WO_d = nc.dram_tensor("WO", [GW, D_], bf16, kind="ExternalInput")
    TR_d = nc.dram_tensor("TR", [128, 256], bf16, kind="ExternalInput")
    ON_d = nc.dram_tensor("ON", [128, 512], bf16, kind="ExternalInput")
    Y_d = nc.dram_tensor("Y", [S_, D_], bf16, kind="ExternalOutput")

    Exp = mybir.ActivationFunctionType.Exp

    with tile.TileContext(nc) as tc:
        with tc.tile_pool(name="const", bufs=1) as const_pool, \
             tc.tile_pool(name="big", bufs=1) as big_pool, \
             tc.tile_pool(name="xw", bufs=2) as xw_pool, \
             tc.tile_pool(name="pt", bufs=4) as pt_pool, \
             tc.tile_pool(name="ctxn", bufs=12) as ctxn_pool, \
             tc.tile_pool(name="rec", bufs=4) as rec_pool, \
             tc.tile_pool(name="ys", bufs=4) as ys_pool, \
             tc.tile_pool(name="ps_stp", bufs=2, space="PSUM") as ps_stp, \
             tc.tile_pool(name="ps_ctx", bufs=2, space="PSUM") as ps_ctx, \
             tc.tile_pool(name="ps_misc", bufs=2, space="PSUM") as ps_misc:

            # ---- persistent tensors ----
            QT_t = big_pool.tile([128, NP, S_], bf16)      # Q^T  [dout, s]
            KTT = big_pool.tile([128, NP, S_], bf16)       # K^T  [dout, s]
            # V per (key-chunk, head): [ones(64) | V(64)]
            V_t = big_pool.tile([128, NKC, 2 * NP, 128], bf16)

            # PE ramp warm-up: the cost model runs the PE at reduced clock
            # until it has been busy ~3us; a burst of dummy matmuls on
            # zeroed SBUF warms it up while the first input DMAs are still
            # in flight.
            wu = const_pool.tile([128, 128], bf16, tag="wu")
            nc.vector.memset(wu, 0)
            psw = ps_misc.tile([128, 512], f32, tag="ps")
            for r in range(12):
                nc.tensor.matmul(psw[:, 0:128], lhsT=wu, rhs=wu,
                                 start=(r == 0), stop=(r == 11),
                                 skip_group_check=True)

            WQ_t = const_pool.tile([128, KT, GW], bf16, tag="wq")
            WK_t = const_pool.tile([128, KT, GW], bf16, tag="wk")
            WV_t = const_pool.tile([128, KT, GW], bf16, tag="wv")
            WO_t = const_pool.tile([128, NP, D_], bf16, tag="wo")
            # TRI[p, i, j] = 1 iff p <= j: keep-mask for the causal diagonal
            # block, applied to the exp'd scores on the DVE
            tri = const_pool.tile([128, 2, 128], bf16, tag="tri")

            XT_r = XT_d.rearrange("(kt p) s -> p kt s", p=128)
            WQ_r = WQ_d.rearrange("(kt p) n -> p kt n", p=128)
            WK_r = WK_d.rearrange("(kt p) n -> p kt n", p=128)
            WV_r = WV_d.rearrange("(kt p) n -> p kt n", p=128)

            def dma_x(st):
                # halves rather than per-kt chunks: the shared HWDGE issue
                # pipe (~625ns per DMA) costs more than the transfer here
                xt = xw_pool.tile([128, KT, 512], bf16, tag="xt",
                                  name=f"xt_{st}")
                h = KT // 2
                nc.sync.dma_start(xt[:, 0:h],
                                  XT_r[:, 0:h, st * 512:(st + 1) * 512])
                nc.sync.dma_start(xt[:, h:KT],
                                  XT_r[:, h:KT, st * 512:(st + 1) * 512])
                return xt

            # startup DMAs. The X/WK chunk pairs feeding the first
            # projections are interleaved across the SP and Activation
            # sequencers; everything non-critical goes through the gpsimd
            # (SWDGE) path so the shared HWDGE issue pipe stays clear.
            xts = {}
            xt0 = xw_pool.tile([128, KT, 512], bf16, tag="xt", name="xt_0")
            xts[0] = xt0
            cuts = (0, 1, max(2, KT // 2), KT) if KT > 2 else (0, 1, KT)
            for lo, hi in zip(cuts, cuts[1:]):
                nc.sync.dma_start(xt0[:, lo:hi], XT_r[:, lo:hi, 0:512])
                nc.scalar.dma_start(WK_t[:, lo:hi], WK_r[:, lo:hi])
            nc.scalar.dma_start(WQ_t, WQ_r)
            nc.gpsimd.dma_start(tri, TR_d.rearrange("p (i j) -> p i j", j=128))
            for kt in range(KT):
                nc.gpsimd.dma_start(WV_t[:, kt], WV_r[:, kt])
            ON_r = ON_d.rearrange("p (h e) -> p h e", e=64)
            # ones blocks for the first key tile only; the rest (and WO) are
            # deferred into the main loop so they don't steal DMA bandwidth
            # from the startup-critical X/W chunks.
            for kc in range(4):
                nc.gpsimd.dma_start(V_t[:, kc, :, 0:64], ON_r)

            def emit_deferred_dmas(qt, c):
                if qt == 0 and c == 0:
                    nc.gpsimd.dma_start(
                        WO_t, WO_d.rearrange("(c p) n -> p c n", p=128))
                if qt == 0 and c < 3 and NKC > 4:
                    for kc in range(4 + 4 * c, min(NKC, 8 + 4 * c)):
                        nc.gpsimd.dma_start(V_t[:, kc, :, 0:64], ON_r)

            # ---- projection emitters (also used as pipeline filler) ----
            def emit_kqproj(wt, outt, st, c):
                ps = ps_misc.tile([128, 512], f32, tag="ps")
                for kt in range(KT):
                    nc.tensor.matmul(
                        ps, lhsT=wt[:, kt, c * 128:(c + 1) * 128],
                        rhs=xts[st][:, kt, :],
                        start=(kt == 0), stop=(kt == KT - 1))
                nc.vector.tensor_copy(
                    out=outt[:, c, st * 512:(st + 1) * 512], in_=ps)

            def emit_vproj(st, sc):
                kc = st * 4 + sc
                ps = ps_misc.tile([128, 512], f32, tag="ps")
                for kt in range(KT):
                    nc.tensor.matmul(
                        ps, lhsT=xts[st][:, kt, sc * 128:(sc + 1) * 128],
                        rhs=WV_t[:, kt, :],
                        start=(kt == 0), stop=(kt == KT - 1))
                nc.vector.tensor_copy(
                    out=V_t[:, kc, :, 64:128],
                    in_=ps.rearrange("p (h d) -> p h d", d=64))

            OW = min(512, D_)
            NOUT = D_ // OW

            def emit_oproj(qt, ctxn, sss, vector_only=False):
                for ss in sss:
                    for n in range(NOUT):
                        yp = ps_misc.tile([128, 512], f32, tag="ps")
                        for c in range(NP):
                            nc.tensor.matmul(
                                yp[:, 0:OW],
                                lhsT=ctxn[c][:, ss * 128:(ss + 1) * 128],
                                rhs=WO_t[:, c, n * OW:(n + 1) * OW],
                                start=(c == 0), stop=(c == NP - 1))
                        ys = ys_pool.tile([128, OW], bf16, tag="ys")
                        if (ss + n) % 2 == 0 and not vector_only:
                            nc.scalar.copy(out=ys, in_=yp[:, 0:OW])
                        else:
                            nc.vector.tensor_copy(out=ys, in_=yp[:, 0:OW])
                        nc.sync.dma_start(
                            Y_d[qt * 512 + ss * 128: qt * 512 + (ss + 1) * 128,
                                n * OW:(n + 1) * OW],
                            ys)

            # ---- startup projections: K/V for st0, Q for qt0 (all later
            # K/V/Q projections are emitted just-in-time inside the windows
            # that consume them, as PE filler against the Act-bound exp) ----
            for c in range(NP):
                emit_kqproj(WK_t, KTT, 0, c)
            for c in range(NP):
                emit_kqproj(WQ_t, QT_t, 0, c)
            for sc in range(4):
                emit_vproj(0, sc)

            # ---- attention per (qt, c) ----
            def emit_attention(qt, c):
                qs = qt * 512
                nkc = 4 * qt + 4
                ctx = [ps_ctx.tile([128, 512], f32, tag="ctx",
                                   name=f"ctx{i}_{qt}_{c}")
                       for i in range(2)]
                for kc in range(nkc):
                    jp = kc - 4 * qt
                    trim = 128 * jp if jp >= 0 else 0
                    stp = ps_stp.tile([128, 2, 512], f32, tag="stp")
                    for i in (0, 1):
                        nc.tensor.matmul(
                            stp[:, i, trim:512],
                            lhsT=KTT[64 * i:64 * i + 64, c,
                                     kc * 128:(kc + 1) * 128],
                            rhs=QT_t[64 * i:64 * i + 64, c,
                                     qs + trim:qs + 512],
                            start=True, stop=True,
                            skip_group_check=True)
                    pt = pt_pool.tile([128, 2, 512], bf16, tag="pt")
                    nc.scalar.activation(
                        pt[:, :, trim:512], stp[:, :, trim:512],
                        Exp, scale=SM_SCALE)
                    if jp >= 0:
                        # zero the upper triangle of the diagonal block
                        nc.vector.tensor_mul(
                            pt[:, :, trim:trim + 128],
                            pt[:, :, trim:trim + 128], tri)
                    for i in (0, 1):
                        nc.tensor.matmul(
                            ctx[i][:, trim:512],
                            lhsT=V_t[:, kc, 2 * c + i, :],
                            rhs=pt[:, i, trim:512],
                            start=(kc == 0), stop=(kc == nkc - 1),
                            skip_group_check=True)
                # normalization: rows 0-63 hold the denominator replicated,
                # rows 64-127 the context. For the very last (qt, c) the
                # work is split into column halves so the final out-proj can
                # start on the first half while the second drains.
                ctxn = ctxn_pool.tile([128, 512], bf16, tag="ctxn",
                                      name=f"ctxn_{qt}_{c}")
                halves = ((0, 256), (256, 512)) if (
                    qt == NQT - 1 and c == NP - 1) else ((0, 512),)
                for lo, hi in halves:
                    for i in (0, 1):
                        rec = rec_pool.tile([64, 512], f32, tag="rec")
                        nc.vector.reciprocal_approx_fast(
                            out=rec[:, lo:hi], in_=ctx[i][0:64, lo:hi])
                        nc.vector.tensor_mul(
                            ctxn[64 * i:64 * i + 64, lo:hi],
                            ctx[i][64:128, lo:hi], rec[:, lo:hi])
                return ctxn

            # K/V(st) are only consumed by window st's last 4 chunks, so for
            # st >= 1 they are emitted just-in-time INSIDE window st as PE
            # filler against the Act-bound exp stream. Q(qt) must be ready at
            # window qt's start, so it runs one window ahead.
            # All filler work (JIT K/V/Q projections, deferred out-proj) is
            # emitted DE-prioritized: the per-engine ready heaps then pick it
            # only when the attention stream is stalled on a dependency, so
            # the filler self-rations across the exp-wait bubbles instead of
            # being greedily consumed at each window's start. Out-projs are
            # deferred TWO windows so the (deficit-heaviest) last window gets
            # a double helping of filler.
            pending = []         # [(qt, [ctxn per c])] awaiting out-proj
            for qt in range(NQT):
                ctxns = []
                due = []
                if qt == NQT - 1:
                    due = pending
                elif pending and pending[0][0] <= qt - 2:
                    due = [pending.pop(0)]
                for c in range(NP):
                    emit_deferred_dmas(qt, c)
                    with tc.high_priority(offset=-(10 ** 6)):
                        if qt > 0:
                            if c == 0:
                                for sc in range(4):
                                    emit_vproj(qt, sc)
                            emit_kqproj(WK_t, KTT, qt, c)
                    ctxns.append(emit_attention(qt, c))
                    with tc.high_priority(offset=-(10 ** 6)):
                        if qt + 1 < NQT:
                            if c == 0:
                                xts[qt + 1] = dma_x(qt + 1)
                            emit_kqproj(WQ_t, QT_t, qt + 1, c)
                        for dqt, dctxns in due:
                            emit_oproj(dqt, dctxns, (c,))
                pending.append((qt, ctxns))
            emit_oproj(pending[-1][0], pending[-1][1], (0, 1, 2, 3),
                       vector_only=True)

    nc.compile()
    return nc


_NC_CACHE = {}


def _get_nc():
    key = (S, D)
    if key not in _NC_CACHE:
        _NC_CACHE[key] = build_mha_kernel(S, D)
    return _NC_CACHE[key]


def make_consts():
    r = np.arange(128)
    # keep-mask for the causal diagonal block: TRI[p, j] = 1 iff p <= j,
    # duplicated for both heads of a pair
    tri1 = (r[:, None] <= r[None, :]).astype(BF)
    tr = np.concatenate([tri1, tri1], axis=1)
    on = np.ones((128, 512), dtype=BF)
    return tr, on


def shard_inputs(X, Wq, Wk, Wv, Wo):
    """Build the 8 per-core input maps from full inputs."""
    X = np.asarray(X, dtype=np.float32)
    Wq = np.asarray(Wq, dtype=np.float32)
    Wk = np.asarray(Wk, dtype=np.float32)
    Wv = np.asarray(Wv, dtype=np.float32)
    Wo = np.asarray(Wo, dtype=np.float32)
    tr, on = make_consts()
    in_maps = []
    for c in range(NCORES):
        b, g = c // 2, c % 2
        in_maps.append({
            "XT": np.ascontiguousarray(X[b].T).astype(BF),
            "WQ": np.ascontiguousarray(Wq[:, g * GW:(g + 1) * GW]).astype(BF),
            "WK": np.ascontiguousarray(Wk[:, g * GW:(g + 1) * GW]).astype(BF),
            "WV": np.ascontiguousarray(Wv[:, g * GW:(g + 1) * GW]).astype(BF),
            "WO": np.ascontiguousarray(Wo[g * GW:(g + 1) * GW, :]).astype(BF),
            "TR": tr, "ON": on,
        })
    return in_maps


def kernel(X, Wq, Wk, Wv, Wo, bo):
    from concourse.bass_utils import run_bass_kernel_spmd

    nc = _get_nc()
    in_maps = shard_inputs(X, Wq, Wk, Wv, Wo)
    res = run_bass_kernel_spmd(nc, in_maps, core_ids=list(range(NCORES)))
    bo = np.asarray(bo, dtype=np.float32)
    Y = np.empty((B, S, D), dtype=np.float32)
    for b in range(B):
        Y[b] = (res.results[2 * b]["Y"].astype(np.float32)
                + res.results[2 * b + 1]["Y"].astype(np.float32) + bo)
    return Y
